# revision 1
# baseline (speedup 1.0000x reference)
"""CharacterLanguageModel LSTM kernel for 8 TRN2 NeuronCores.

8-way tensor-parallel over the hidden dim H=1024: each core owns one 128-row
chunk of h/c and the matching 512 gate rows (order i,f,o,g). Per timestep:
  1. gates_r = sum_d WhhT[d] applied to h-slot d  +  U_r[char]  (one-hot tile)
     -- 9 fp32r matmuls, h slots are the stationary operands in [128, n] layout
  2. sigmoid/tanh on ACT, c/h update on DVE for the active prefix n_t
  3. PE-transpose h_new [n,128] -> [128,n], copy into the slot history, and
     remote-DMA-broadcast it to the 7 peers (XOR slot addressing keeps the
     SPMD program identical; per-core weight layouts are XOR-permuted on host)
Embedding lookup + input projection + both biases are folded into a host-built
table U = emb @ Wih.T + bih + bhh applied via the one-hot contraction tile.
Ragged lengths: batch sorted descending by length, active batch is a prefix,
every instruction touches only the first n_t columns; outputs beyond a
sequence's length stay zero via delta-memsets of the slot history.
Decoder runs on the slot history once per S-step chunk in PE idle time; all
cores compute the full decode (host reads core 0's).
"""

import numpy as np

import concourse.bass as bass
import concourse.mybir as mybir
from concourse import library_config
from concourse.library_overlay import lower_extended_insts
from concourse.bass_utils import run_bass_kernel_spmd

P = 128
NCORES = 8
GATE = 512
S = 16
DEPTH = 2
NLOADS = 7
F32 = mybir.dt.float32
AF = mybir.ActivationFunctionType


def r32(ap):
    return ap.bitcast(mybir.dt.float32r)


def build_nc(Teff, n_sched, B=64):
    assert B == 64
    NCH = (Teff + S - 1) // S
    nc = bass.Bass()

    whh_in = nc.dram_tensor("whh", [P, NCORES * GATE], F32, kind="ExternalInput")
    u_in = nc.dram_tensor("u", [100, GATE], F32, kind="ExternalInput")
    oh_in = nc.dram_tensor("oh", [NCH, 100, S * B], F32, kind="ExternalInput")
    wdec_in = nc.dram_tensor("wdec", [P, NCORES * 100], F32, kind="ExternalInput")
    bdec_in = nc.dram_tensor("bdec", [100, 1], F32, kind="ExternalInput")
    eye_in = nc.dram_tensor("eye", [64, 64], F32, kind="ExternalInput")
    outh = nc.dram_tensor("outh", [NCH, P, S * B], F32, kind="ExternalOutput")
    dec_out = nc.dram_tensor("dec", [NCH, 100, S * B], F32, kind="ExternalOutput")
    cn_out = nc.dram_tensor("cn", [64, P], F32, kind="ExternalOutput")

    from contextlib import ExitStack
    with ExitStack() as ctx:
        block = ctx.enter_context(nc.Block())
        sems = [ctx.enter_context(nc.semaphore(nm)) for nm in (
            "dmasem", "rsem", "lsem", "psem", "mmsem", "trsem", "actsem",
            "dvesem", "decsem", "deccpsem", "outsem", "initsem")]
        (dmasem, rsem, lsem, psem, mmsem, trsem, actsem,
         dvesem, decsem, deccpsem, outsem, initsem) = sems
        sb = lambda nm, shape: ctx.enter_context(nc.sbuf_tensor(nm, shape, F32))
        ps = lambda nm, shape: ctx.enter_context(nc.psum_tensor(nm, shape, F32))
        whh_sb = sb("whh_sb", [P, NCORES, GATE])
        u_sb = sb("u_sb", [100, GATE])
        oh_sb = sb("oh_sb", [100, 2, S, B])
        slot_sb = sb("slot_sb", [P, NCORES, DEPTH, S, B])
        wdec_sb = sb("wdec_sb", [P, NCORES, 100])
        bdec_sb = sb("bdec_sb", [100, 1])
        eye_sb = sb("eye_sb", [64, 64])
        c_sb = sb("c_sb", [64, P])
        sg_sb = sb("sg_sb", [64, 384])
        g_sb = sb("g_sb", [64, P])
        tc_sb = sb("tc_sb", [64, P])
        tmp_sb = sb("tmp_sb", [64, P])
        hrow_sb = sb("hrow_sb", [64, P])
        decstage = sb("decstage", [100, S * B])
        gates_a = ps("gates_a", [64, GATE])
        gates_b = ps("gates_b", [64, GATE])
        ht_a = ps("ht_a", [P, 64])
        ht_b = ps("ht_b", [P, 64])
        dec_a = ps("dec_a", [100, 512])
        dec_b = ps("dec_b", [100, 512])
        gates_ps = [gates_a, gates_b]
        ht_ps = [ht_a, ht_b]
        dec_ps = [dec_a, dec_b]

        def blk(t):
            return (t // S) % DEPTH, t % S

        def slot_flat(d, half):
            # [P, S*B] contiguous view of one slot half
            return slot_sb[:, d, half, :, :].rearrange("p s b -> p (s b)")

        def emit_decode(tensor, c):
            # decoded.T[:, S*B cols] for chunk c, in two 512-col psum groups
            tensor.wait_ge(deccpsem, 2 * c)  # previous chunk copied out of psum
            for hc in range(2):
                cols = slot_flat(0, c % DEPTH)  # placeholder for slicing below
                for d in range(NCORES):
                    mm = tensor.matmul(
                        dec_ps[hc][:, :],
                        r32(wdec_sb[:, d, :]),
                        r32(slot_flat(d, c % DEPTH)[:, hc * 512:(hc + 1) * 512]),
                        start=(d == 0), stop=(d == NCORES - 1),
                    )
                    if d == NCORES - 1:
                        mm.then_inc(decsem, 1)

        def emit_deccopy(scalar, c):
            scalar.wait_ge(decsem, 2 * (c + 1))
            if c >= 1:
                scalar.wait_ge(outsem, 32 * c)  # previous decstage DMA done
            for hc in range(2):
                scalar.activation(
                    decstage[:, hc * 512:(hc + 1) * 512],
                    dec_ps[hc][:, :],
                    AF.Identity, bias=bdec_sb[:, 0:1],
                ).then_inc(deccpsem, 1)

        # ---------------- SYNC: all DMA ----------------
        @block.sync
        def _(sync):
            sync.dma_start(
                whh_sb[:, :, :], whh_in[:, :].rearrange("p (d g) -> p d g", g=GATE)
            ).then_inc(dmasem, 16)
            sync.dma_start(u_sb[:, :], u_in[:, :]).then_inc(dmasem, 16)
            sync.dma_start(
                wdec_sb[:, :, :], wdec_in[:, :].rearrange("p (d v) -> p d v", v=100)
            ).then_inc(dmasem, 16)
            sync.dma_start(bdec_sb[:, :], bdec_in[:, :]).then_inc(dmasem, 16)
            sync.dma_start(eye_sb[:, :], eye_in[:, :]).then_inc(dmasem, 16)
            sync.dma_start(
                oh_sb[:, 0, :, :].rearrange("v s b -> v (s b)"), oh_in[0, :, :]
            ).then_inc(dmasem, 16)
            if NCH > 1:
                sync.dma_start(
                    oh_sb[:, 1, :, :].rearrange("v s b -> v (s b)"), oh_in[1, :, :]
                ).then_inc(dmasem, 16)
            else:
                sync.sem_inc(dmasem, 16)
            for c in range(NCH):
                if c >= 2:
                    sync.wait_ge(mmsem, (c - 1) * S)
                    sync.dma_start(
                        oh_sb[:, c % 2, :, :].rearrange("v s b -> v (s b)"),
                        oh_in[c, :, :],
                    ).then_inc(dmasem, 16)
                t_last = min((c + 1) * S, Teff)
                sync.wait_ge(actsem, 4 * t_last)
                sync.dma_start(outh[c, :, :], slot_flat(0, c % DEPTH)).then_inc(outsem, 16)
                sync.wait_ge(deccpsem, 2 * (c + 1))
                sync.dma_start(dec_out[c, :, :], decstage[:, :]).then_inc(outsem, 16)
            sync.wait_ge(dvesem, 2 * Teff)
            sync.dma_start(cn_out[:, :], c_sb[:, :]).then_inc(outsem, 16)

        # ---------------- GPSIMD: exchange ----------------
        @block.gpsimd
        def _(gpsimd):
            gpsimd.load_library(library_config.remote_dma)
            gpsimd.wait_ge(initsem, 1)
            for d in range(1, NCORES):
                rd = [None] * 8
                rd[d] = (0, d)
                gpsimd.remote_sem_update_broadcast(rsem, lsem, rdests=rd).then_inc(psem, 1)
            gpsimd.wait_ge(psem, 7)
            gpsimd.trigger_dma(count=7)
            gpsimd.wait_ge(rsem, 14)  # all peers past their init memset
            for t in range(Teff):
                half, s = blk(t)
                n = n_sched[t]
                gpsimd.wait_ge(lsem, 112 * (t + 1))
                for d in range(1, NCORES):
                    rd = [None] * 8
                    rd[d] = (0, d)
                    gpsimd.remote_dma_broadcast(
                        slot_sb[:, d, half, s, 0:n],
                        slot_sb[:, 0, half, s, 0:n],
                        rsem, lsem, rdests=rd,
                    ).then_inc(psem, 1)
                gpsimd.wait_ge(psem, 7 * (t + 2))
                gpsimd.wait_ge(actsem, 4 * t + 4)
                gpsimd.trigger_dma(count=7)

        # ---------------- PE ----------------
        @block.tensor
        def _(tensor):
            tensor.wait_ge(dmasem, 16 * NLOADS)
            for t in range(Teff):
                half, s = blk(t)
                n = n_sched[t]
                gp = gates_ps[t % 2]
                oh_buf = (t // S) % 2
                if t == 0:
                    tensor.matmul(
                        gp[0:n, :], r32(oh_sb[:, 0, 0, 0:n]), r32(u_sb[:, :]),
                        start=True, stop=True,
                    ).then_inc(mmsem, 1)
                else:
                    phalf, ps = blk(t - 1)
                    tensor.wait_ge(actsem, 4 * (t - 1) + 4)  # own slot(t-1)
                    tensor.matmul(
                        gp[0:n, :], r32(slot_sb[:, 0, phalf, ps, 0:n]),
                        r32(whh_sb[:, 0, :]), start=True, stop=False,
                    )
                    tensor.matmul(
                        gp[0:n, :], r32(oh_sb[:, oh_buf, s, 0:n]),
                        r32(u_sb[:, :]), start=False, stop=False,
                    )
                    tensor.wait_ge(rsem, 14 * (t + 1))  # peers' slots(t-1)
                    for d in range(1, NCORES):
                        mm = tensor.matmul(
                            gp[0:n, :], r32(slot_sb[:, d, phalf, ps, 0:n]),
                            r32(whh_sb[:, d, :]),
                            start=False, stop=(d == NCORES - 1),
                        )
                        if d == NCORES - 1:
                            mm.then_inc(mmsem, 1)
                tensor.wait_ge(dvesem, 2 * t + 2)
                tensor.transpose(
                    ht_ps[t % 2][:, 0:n], hrow_sb[0:n, :], eye_sb[0:n, 0:n]
                ).then_inc(trsem, 1)
                if t % S == 0 and t >= S:
                    emit_decode(tensor, t // S - 1)
            tensor.wait_ge(actsem, 4 * Teff)
            tensor.wait_ge(rsem, 14 * (Teff + 1))
            emit_decode(tensor, NCH - 1)

        # ---------------- ACT ----------------
        @block.scalar
        def _(scalar):
            for t in range(Teff):
                half, s = blk(t)
                n = n_sched[t]
                gp = gates_ps[t % 2]
                scalar.wait_ge(mmsem, t + 1)
                scalar.activation(sg_sb[0:n, :], gp[0:n, 0:384], AF.Sigmoid).then_inc(actsem, 1)
                scalar.activation(g_sb[0:n, :], gp[0:n, 384:512], AF.Tanh).then_inc(actsem, 1)
                scalar.wait_ge(dvesem, 2 * t + 1)
                scalar.activation(tc_sb[0:n, :], c_sb[0:n, :], AF.Tanh).then_inc(actsem, 1)
                scalar.wait_ge(trsem, t + 1)
                scalar.activation(
                    slot_sb[:, 0, half, s, 0:n], ht_ps[t % 2][:, 0:n], AF.Copy
                ).then_inc(actsem, 1)
                if t % S == 0 and t >= S:
                    emit_deccopy(scalar, t // S - 1)
            emit_deccopy(scalar, NCH - 1)

        # ---------------- DVE ----------------
        @block.vector
        def _(vector):
            vector.memset(slot_sb[:, :, :, :, :], 0.0)
            vector.sem_inc(initsem, 1)
            for t in range(Teff):
                n = n_sched[t]
                vector.wait_ge(actsem, 4 * t + 2)
                if t == 0:
                    vector.tensor_mul(c_sb[0:n, :], sg_sb[0:n, 0:128], g_sb[0:n, :]).then_inc(dvesem, 1)
                else:
                    vector.tensor_mul(tmp_sb[0:n, :], sg_sb[0:n, 0:128], g_sb[0:n, :])
                    vector.tensor_mul(c_sb[0:n, :], sg_sb[0:n, 128:256], c_sb[0:n, :])
                    vector.tensor_add(c_sb[0:n, :], c_sb[0:n, :], tmp_sb[0:n, :]).then_inc(dvesem, 1)
                vector.wait_ge(actsem, 4 * t + 3)
                vector.tensor_mul(hrow_sb[0:n, :], sg_sb[0:n, 256:384], tc_sb[0:n, :]).then_inc(dvesem, 1)
                # delta-memset for chunk c entered 2 steps from now
                if (t + 2) % S == 0:
                    c = (t + 2) // S
                    if 2 <= c < NCH:
                        n_end = n_sched[(c + 1) * S] if (c + 1) * S < Teff else 0
                        n_hi = n_sched[(c - 2) * S]
                        if n_end < n_hi:
                            vector.wait_ge(decsem, 2 * (c - 1))
                            vector.wait_ge(outsem, 32 * (c - 2) + 16)
                            vector.memset(slot_sb[:, :, c % DEPTH, :, n_end:n_hi], 0.0)

    lower_extended_insts(nc)
    return nc


# ======================= host side =======================

def _gate_cols(r, H):
    j = np.arange(P)
    return np.concatenate(
        [0 * H + r * P + j, 1 * H + r * P + j, 3 * H + r * P + j, 2 * H + r * P + j]
    )  # i, f, o, g


def prep_inputs(chars_s, Teff, emb, Wih, Whh, bih, bhh, Wdec, bdec):
    V, E = emb.shape
    H = Whh.shape[1]
    B = chars_s.shape[0]
    NCH = (Teff + S - 1) // S
    U = (emb.astype(np.float64) @ Wih.T.astype(np.float64)) + bih + bhh
    U = U.astype(np.float32)

    oh = np.zeros((NCH * S, V, B), np.float32)
    tt = np.arange(Teff)[:, None]
    bb = np.arange(B)[None, :]
    oh[tt, np.asarray(chars_s[:, :Teff].T), bb] = 1.0
    oh = np.ascontiguousarray(oh.reshape(NCH, S, V, B).transpose(0, 2, 1, 3).reshape(NCH, V, S * B))

    ins = []
    for r in range(NCORES):
        gc = _gate_cols(r, H)
        whhT = np.empty((P, NCORES, GATE), np.float32)
        wdecT = np.empty((P, NCORES, 100), np.float32)
        for d in range(NCORES):
            k = r ^ d
            whhT[:, d, :] = Whh[gc][:, k * P:(k + 1) * P].T
            wdecT[:, d, :] = Wdec[:, k * P:(k + 1) * P].T
        ins.append({
            "whh": np.ascontiguousarray(whhT.reshape(P, NCORES * GATE)),
            "u": np.ascontiguousarray(U[:, gc]),
            "oh": oh,
            "wdec": np.ascontiguousarray(wdecT.reshape(P, NCORES * 100)),
            "bdec": np.ascontiguousarray(bdec.reshape(100, 1).astype(np.float32)),
            "eye": np.eye(64, dtype=np.float32),
        })
    return ins


def assemble(res_list, order, charlens, B, T, H, V, Teff, bdec):
    NCH = (Teff + S - 1) // S
    L = min(Teff, T)
    output = np.zeros((B, T, H), np.float32)
    for r in range(NCORES):
        oh_r = np.asarray(res_list[r]["outh"]).reshape(NCH, P, S, B)
        flat = oh_r.transpose(0, 2, 3, 1).reshape(NCH * S, B, P)  # [t, b, p]
        output[order, :L, r * P:(r + 1) * P] = flat[:L].transpose(1, 0, 2)

    decoded = np.empty((B, T, V), np.float32)
    decoded[:] = np.asarray(bdec, np.float32)[None, None, :]
    dec_r = np.asarray(res_list[0]["dec"]).reshape(NCH, 100, S, B)
    dflat = dec_r.transpose(0, 2, 3, 1).reshape(NCH * S, B, 100)
    decoded[order, :L, :] = dflat[:L, :, :V].transpose(1, 0, 2)

    lens = np.asarray(charlens).astype(np.int64)
    hn = output[np.arange(B), np.maximum(lens - 1, 0), :].astype(np.float32)

    cn = np.zeros((B, H), np.float32)
    for r in range(NCORES):
        cn[order, r * P:(r + 1) * P] = np.asarray(res_list[r]["cn"])
    return output, hn, cn, decoded


def kernel(chars, charlens, emb, Wih, Whh, bih, bhh, Wdec, bdec):
    chars = np.asarray(chars)
    charlens = np.asarray(charlens)
    emb = np.asarray(emb, np.float32)
    Wih = np.asarray(Wih, np.float32)
    Whh = np.asarray(Whh, np.float32)
    bih = np.asarray(bih, np.float32)
    bhh = np.asarray(bhh, np.float32)
    Wdec = np.asarray(Wdec, np.float32)
    bdec = np.asarray(bdec, np.float32)

    B, T = chars.shape
    H = Whh.shape[1]
    V = emb.shape[0]
    order = np.argsort(-charlens, kind="stable")
    lens_s = charlens[order]
    chars_s = chars[order]
    Teff = int(lens_s[0])
    n_sched = [int(np.sum(lens_s > t)) for t in range(Teff)]

    nc = build_nc(Teff, n_sched, B=B)
    in_maps = prep_inputs(chars_s, Teff, emb, Wih, Whh, bih, bhh, Wdec, bdec)
    res = run_bass_kernel_spmd(nc, in_maps, core_ids=list(range(NCORES)))
    return assemble(res.results, order, charlens, B, T, H, V, Teff, bdec)
